# revision 5
# baseline (speedup 1.0000x reference)
"""Mistral MoE layer (H=2048, F=8192, E=8, top-2) on 8 Trainium2 NeuronCores.

Strategy (expert parallelism, per the sharding hint):
  - Host computes the (tiny) gate: logits = x @ gate_w, top-2, softmax.
    This is 0.004% of the FLOPs; the expert FFNs dominate.
  - Host "all-to-all dispatch": tokens are gathered per expert into a
    padded capacity buffer of C tokens (C = max expert load rounded up).
  - Core e runs expert e's SwiGLU FFN over its C tokens:
        y = (silu(x @ w1) * (x @ w3)) @ w2, scaled per-token by the
    combined gate weight.
  - Host "combine": scatter-add each expert's rows back into the output.

Device kernel (per core):
  Stage 1 computes hT/uT = w1/w3-projections in *transposed* form
  [F, C] so stage 2 can use them directly as the stationary matmul
  operand without any on-device transpose.  Matmuls run as float32r
  (FP22 truncated fp32 — full 78.6 TF/s PE rate at moving dim >= 256).
  yT = silu(hT) * uT is stored bf16; stage 2 (down-projection) runs
  bf16 x bf16.  F is processed in groups of G=8 f-tiles; each group's
  partial down-projection output is accumulated via a DRAM bounce
  buffer: out_g = psum * gate_w + out_{g-1} (fused DVE op).
"""

import math

import numpy as np
import ml_dtypes

import concourse.bass as bass
import concourse.mybir as mybir
import concourse.tile as tile
from concourse import bacc
from concourse.bass_utils import run_bass_kernel_spmd

P = 128
H = 2048
F = 8192
E = 8
TOP_K = 2

_kernel_cache: dict = {}

# Test-harness knobs (ignored in normal use): when TRACE is true, the SPMD
# run captures an NTFF profile and the BassKernelResults lands in LAST_RESULT.
TRACE = False
LAST_RESULT = None


def build_expert_kernel(C, H_=H, F_=F, c_chunk=384, ho_chunk=512, G=8):
    """One expert's SwiGLU FFN over C tokens; returns finalized Bacc."""
    f32 = mybir.dt.float32
    f32r = mybir.dt.float32r
    bf16 = mybir.dt.bfloat16

    n_hh = H_ // P          # contraction tiles over hidden dim (stage 1)
    n_f = F_ // P           # f tiles
    n_ct = C // P           # token tiles (stage 2 output partitions)
    n_cch = C // c_chunk    # moving-dim chunks over tokens (stage 1)
    n_ho = H_ // ho_chunk   # moving-dim chunks over hidden dim (stage 2)
    n_groups = n_f // G
    assert C % c_chunk == 0 and C % P == 0 and F_ % (G * P) == 0

    nc = bacc.Bacc("TRN2", target_bir_lowering=False, debug=False)
    xt_d = nc.dram_tensor("xt", [H_, C], f32r, kind="ExternalInput")
    w1_d = nc.dram_tensor("w1", [H_, F_], f32r, kind="ExternalInput")
    w3_d = nc.dram_tensor("w3", [H_, F_], f32r, kind="ExternalInput")
    w2_d = nc.dram_tensor("w2b", [F_, H_], bf16, kind="ExternalInput")
    gw_d = nc.dram_tensor("gws", [P, n_ct], f32, kind="ExternalInput")
    out_d = nc.dram_tensor("out", [C, H_], f32, kind="ExternalOutput")

    xt_r = xt_d[:, :].rearrange("(ho hi) c -> hi ho c", hi=P)
    w1_r = w1_d[:, :].rearrange("(ho hi) f -> hi ho f", hi=P)
    w3_r = w3_d[:, :].rearrange("(ho hi) f -> hi ho f", hi=P)

    with tile.TileContext(nc) as tc:
        with (
            tc.tile_pool(name="persist", bufs=1) as persist,
            tc.tile_pool(name="wpool", bufs=2) as wpool,
            tc.tile_pool(name="ypool", bufs=1) as ypool,
            tc.tile_pool(name="spool", bufs=2) as spool,
            tc.tile_pool(name="opool", bufs=2) as opool,
            tc.tile_pool(name="psum", bufs=1, space="PSUM") as psum,
            tc.tile_pool(name="dram", bufs=1, space="DRAM") as dram,
        ):
            xt_s = persist.tile([P, n_hh, C], f32r, name="xt_s")
            nc.sync.dma_start(xt_s[:], xt_r)
            gw_s = persist.tile([P, n_ct], f32, name="gw_s")
            nc.sync.dma_start(gw_s[:], gw_d[:, :])
            part_d = dram.tile([C, H_], f32, name="part")

            for g in range(n_groups):
                # ---- stage 1: yT[f_tile, :] for the G f-tiles of this group
                yt = ypool.tile([P, G, C], bf16, tag="yt", name="yt")
                for fi in range(G):
                    f = g * G + fi
                    fsl = bass.ts(f, P)
                    w1_t = wpool.tile([P, n_hh, P], f32r, tag="w1t", name="w1_t")
                    nc.sync.dma_start(w1_t[:], w1_r[:, :, fsl])
                    w3_t = wpool.tile([P, n_hh, P], f32r, tag="w3t", name="w3_t")
                    nc.sync.dma_start(w3_t[:], w3_r[:, :, fsl])
                    for ci in range(n_cch):
                        csl = bass.ts(ci, c_chunk)
                        ph = psum.tile([P, c_chunk], f32, tag="ph", bufs=2, name="ph")
                        pu = psum.tile([P, c_chunk], f32, tag="pu", bufs=2, name="pu")
                        for hh in range(n_hh):
                            nc.tensor.matmul(
                                ph[:],
                                w1_t[:, hh, :],
                                xt_s[:, hh, csl],
                                start=(hh == 0),
                                stop=(hh == n_hh - 1),
                            )
                        for hh in range(n_hh):
                            nc.tensor.matmul(
                                pu[:],
                                w3_t[:, hh, :],
                                xt_s[:, hh, csl],
                                start=(hh == 0),
                                stop=(hh == n_hh - 1),
                            )
                        sl = spool.tile([P, c_chunk], f32, tag="sl", name="sl")
                        nc.scalar.activation(
                            sl[:], ph[:], mybir.ActivationFunctionType.Silu
                        )
                        nc.vector.tensor_tensor(
                            yt[:, fi, csl], sl[:], pu[:], mybir.AluOpType.mult
                        )

                # ---- stage 2: partial down-projection for this group
                w2_t = wpool.tile([P, G, H_], bf16, tag="w2t", bufs=1, name="w2_t")
                for fi in range(G):
                    f = g * G + fi
                    nc.sync.dma_start(w2_t[:, fi, :], w2_d[bass.ts(f, P), :])
                for ct in range(n_ct):
                    ctsl = bass.ts(ct, P)
                    po = psum.tile([P, H_], f32, tag="po", bufs=1, name="po")
                    for fi in range(G):
                        for ho in range(n_ho):
                            hosl = bass.ts(ho, ho_chunk)
                            nc.tensor.matmul(
                                po[:, hosl],
                                yt[:, fi, ctsl],
                                w2_t[:, fi, hosl],
                                start=(fi == 0),
                                stop=(fi == G - 1),
                            )
                    ot = opool.tile([P, H_], f32, tag="ot", name="ot")
                    gsl = gw_s[:, ct : ct + 1]
                    if g == 0:
                        nc.vector.tensor_scalar_mul(ot[:], po[:], gsl)
                    else:
                        prev = opool.tile([P, H_], f32, tag="prev", name="prev")
                        nc.sync.dma_start(prev[:], part_d[ctsl, :])
                        nc.vector.scalar_tensor_tensor(
                            ot[:],
                            po[:],
                            gsl,
                            prev[:],
                            mybir.AluOpType.mult,
                            mybir.AluOpType.add,
                        )
                    dst = out_d if g == n_groups - 1 else part_d
                    nc.sync.dma_start(dst[ctsl, :], ot[:])
    nc.finalize()
    return nc


def _route(x, gate_w):
    """Host gate: top-2 + softmax.  Returns (idx per expert, weight per expert)."""
    xs = x.reshape(-1, x.shape[-1])
    logits = xs.astype(np.float32) @ gate_w.astype(np.float32)  # [T, E]
    # top-2 (ties broken by lower index, matching jax.lax.top_k)
    e1 = np.argmax(logits, axis=1)
    l1 = logits[np.arange(len(logits)), e1]
    masked = logits.copy()
    masked[np.arange(len(logits)), e1] = -np.inf
    e2 = np.argmax(masked, axis=1)
    l2 = masked[np.arange(len(logits)), e2]
    # softmax over the two logits
    w_hi = 1.0 / (1.0 + np.exp(l2 - l1))
    w_lo = 1.0 - w_hi
    idxs, gws = [], []
    for e in range(E):
        sel1 = e1 == e
        sel2 = e2 == e
        idx = np.nonzero(sel1 | sel2)[0]
        w = np.where(sel1[idx], w_hi[idx], w_lo[idx]).astype(np.float32)
        idxs.append(idx)
        gws.append(w)
    return xs, idxs, gws


def kernel(x, gate_w, w1, w3, w2):
    x = np.asarray(x)
    gate_w = np.asarray(gate_w)
    w1 = np.asarray(w1)
    w3 = np.asarray(w3)
    w2 = np.asarray(w2)

    xs, idxs, gws = _route(x, gate_w)
    T = xs.shape[0]
    max_load = max(len(i) for i in idxs)
    C = max(384, int(math.ceil(max_load / 384.0)) * 384)

    key = ("k", C)
    if key not in _kernel_cache:
        _kernel_cache[key] = build_expert_kernel(C)
    nc = _kernel_cache[key]

    in_maps = []
    for e in range(E):
        n_e = len(idxs[e])
        xt = np.zeros((H, C), np.float32)
        xt[:, :n_e] = xs[idxs[e]].T
        gwpad = np.zeros(C, np.float32)
        gwpad[:n_e] = gws[e]
        gws_arr = np.ascontiguousarray(gwpad.reshape(C // P, P).T)
        in_maps.append(
            {
                "xt": xt,
                "w1": np.ascontiguousarray(w1[e], dtype=np.float32),
                "w3": np.ascontiguousarray(w3[e], dtype=np.float32),
                "w2b": np.ascontiguousarray(w2[e]).astype(ml_dtypes.bfloat16),
                "gws": gws_arr,
            }
        )

    global LAST_RESULT
    if TRACE:
        try:
            res = run_bass_kernel_spmd(
                nc,
                in_maps,
                core_ids=list(range(E)),
                trace=True,
                trace_cores=list(range(E)),
            )
        except Exception as exc:
            import traceback

            print("TRACE FAILED:", exc)
            traceback.print_exc()
            res = run_bass_kernel_spmd(nc, in_maps, core_ids=list(range(E)))
    else:
        res = run_bass_kernel_spmd(nc, in_maps, core_ids=list(range(E)))
    LAST_RESULT = res
    results = res.results

    out_flat = np.zeros((T, H), np.float32)
    for e in range(E):
        n_e = len(idxs[e])
        out_flat[idxs[e]] += results[e]["out"][:n_e]
    return out_flat.reshape(x.shape)


# revision 6
# speedup vs baseline: 1.2460x; 1.2460x over previous
"""Mistral MoE layer (H=2048, F=8192, E=8, top-2) on 8 Trainium2 NeuronCores.

Strategy (expert parallelism, per the sharding hint):
  - Host computes the (tiny) gate: logits = x @ gate_w, top-2, softmax.
    This is 0.004% of the FLOPs; the expert FFNs dominate.
  - Host "all-to-all dispatch": tokens are gathered per expert into a
    padded capacity buffer of C tokens (C = max expert load rounded up).
  - Core e runs expert e's SwiGLU FFN over its C tokens:
        y = (silu(x @ w1) * (x @ w3)) @ w2, scaled per-token by the
    combined gate weight.
  - Host "combine": scatter-add each expert's rows back into the output.

Device kernel (per core):
  Stage 1 computes hT/uT = w1/w3-projections in *transposed* form
  [F, C] so stage 2 can use them directly as the stationary matmul
  operand without any on-device transpose.  Matmuls run as float32r
  (FP22 truncated fp32 — full 78.6 TF/s PE rate at moving dim >= 256).
  yT = silu(hT) * uT is stored bf16; stage 2 (down-projection) runs
  bf16 x bf16.  F is processed in groups of G=8 f-tiles; each group's
  partial down-projection output is accumulated via a DRAM bounce
  buffer: out_g = psum * gate_w + out_{g-1} (fused DVE op).
"""

import math

import numpy as np
import ml_dtypes

import concourse.bass as bass
import concourse.mybir as mybir
import concourse.tile as tile
from concourse import bacc
from concourse.bass_utils import run_bass_kernel_spmd

P = 128
H = 2048
F = 8192
E = 8
TOP_K = 2

_kernel_cache: dict = {}

# Test-harness knobs (ignored in normal use): when TRACE is true, the SPMD
# run captures an NTFF profile and the BassKernelResults lands in LAST_RESULT.
TRACE = False
LAST_RESULT = None


def build_expert_kernel(C, H_=H, F_=F, c_chunk=384, ho_chunk=512, G=8):
    """One expert's SwiGLU FFN over C tokens; returns finalized Bacc."""
    f32 = mybir.dt.float32
    f32r = mybir.dt.float32r
    bf16 = mybir.dt.bfloat16

    n_hh = H_ // P          # contraction tiles over hidden dim (stage 1)
    n_f = F_ // P           # f tiles
    n_ct = C // P           # token tiles (stage 2 output partitions)
    n_cch = C // c_chunk    # moving-dim chunks over tokens (stage 1)
    n_ho = H_ // ho_chunk   # moving-dim chunks over hidden dim (stage 2)
    n_groups = n_f // G
    assert C % c_chunk == 0 and C % P == 0 and F_ % (G * P) == 0

    nc = bacc.Bacc("TRN2", target_bir_lowering=False, debug=False)
    xt_d = nc.dram_tensor("xt", [H_, C], f32r, kind="ExternalInput")
    w1_d = nc.dram_tensor("w1", [H_, F_], f32r, kind="ExternalInput")
    w3_d = nc.dram_tensor("w3", [H_, F_], f32r, kind="ExternalInput")
    w2_d = nc.dram_tensor("w2b", [F_, H_], bf16, kind="ExternalInput")
    gw_d = nc.dram_tensor("gws", [P, n_ct], f32, kind="ExternalInput")
    out_d = nc.dram_tensor("out", [C, H_], f32, kind="ExternalOutput")

    xt_r = xt_d[:, :].rearrange("(ho hi) c -> hi ho c", hi=P)
    w1_r = w1_d[:, :].rearrange("(ho hi) f -> hi ho f", hi=P)
    w3_r = w3_d[:, :].rearrange("(ho hi) f -> hi ho f", hi=P)

    with tile.TileContext(nc) as tc:
        with (
            tc.tile_pool(name="persist", bufs=1) as persist,
            tc.tile_pool(name="wpool", bufs=2) as wpool,
            tc.tile_pool(name="ypool", bufs=1) as ypool,
            tc.tile_pool(name="spool", bufs=2) as spool,
            tc.tile_pool(name="opool", bufs=2) as opool,
            tc.tile_pool(name="psum", bufs=1, space="PSUM") as psum,
            tc.tile_pool(name="dram", bufs=1, space="DRAM") as dram,
        ):
            xt_s = persist.tile([P, n_hh, C], f32r, name="xt_s")
            nc.sync.dma_start(xt_s[:], xt_r)
            gw_s = persist.tile([P, n_ct], f32, name="gw_s")
            nc.sync.dma_start(gw_s[:], gw_d[:, :])
            part_d = dram.tile([C, H_], f32, name="part")

            for g in range(n_groups):
                # ---- stage 1: yT[f_tile, :] for the G f-tiles of this group
                yt = ypool.tile([P, G, C], bf16, tag="yt", name="yt")
                for fi in range(G):
                    f = g * G + fi
                    fsl = bass.ts(f, P)
                    w1_t = wpool.tile([P, n_hh, P], f32r, tag="w1t", name="w1_t")
                    nc.sync.dma_start(w1_t[:], w1_r[:, :, fsl])
                    w3_t = wpool.tile([P, n_hh, P], f32r, tag="w3t", name="w3_t")
                    nc.sync.dma_start(w3_t[:], w3_r[:, :, fsl])
                    for ci in range(n_cch):
                        csl = bass.ts(ci, c_chunk)
                        ph = psum.tile([P, c_chunk], f32, tag="ph", bufs=2, name="ph")
                        pu = psum.tile([P, c_chunk], f32, tag="pu", bufs=2, name="pu")
                        for hh in range(n_hh):
                            nc.tensor.matmul(
                                ph[:],
                                w1_t[:, hh, :],
                                xt_s[:, hh, csl],
                                start=(hh == 0),
                                stop=(hh == n_hh - 1),
                            )
                        for hh in range(n_hh):
                            nc.tensor.matmul(
                                pu[:],
                                w3_t[:, hh, :],
                                xt_s[:, hh, csl],
                                start=(hh == 0),
                                stop=(hh == n_hh - 1),
                            )
                        sl = spool.tile([P, c_chunk], f32, tag="sl", name="sl")
                        nc.scalar.activation(
                            sl[:], ph[:], mybir.ActivationFunctionType.Silu
                        )
                        nc.vector.tensor_tensor(
                            yt[:, fi, csl], sl[:], pu[:], mybir.AluOpType.mult
                        )

                # ---- stage 2: partial down-projection for this group
                w2_t = wpool.tile([P, G, H_], bf16, tag="w2t", bufs=1, name="w2_t")
                for fi in range(G):
                    f = g * G + fi
                    nc.sync.dma_start(w2_t[:, fi, :], w2_d[bass.ts(f, P), :])
                # two ho-half passes so the PSUM tile is 2 banks and can be
                # double-buffered (next ct's matmuls overlap this ct's DVE read)
                n_half = n_ho // 2 if n_ho >= 2 else 1
                half_w = n_half * ho_chunk
                for ct in range(n_ct):
                    ctsl = bass.ts(ct, P)
                    ot = opool.tile([P, H_], f32, tag="ot", name="ot")
                    gsl = gw_s[:, ct : ct + 1]
                    prev = None
                    if g > 0:
                        prev = opool.tile([P, H_], f32, tag="prev", name="prev")
                        nc.sync.dma_start(prev[:], part_d[ctsl, :])
                    for hf in range(n_ho // n_half):
                        po = psum.tile([P, half_w], f32, tag="po", bufs=2, name="po")
                        for fi in range(G):
                            for ho in range(n_half):
                                hosl = bass.ts(hf * n_half + ho, ho_chunk)
                                nc.tensor.matmul(
                                    po[:, bass.ts(ho, ho_chunk)],
                                    yt[:, fi, ctsl],
                                    w2_t[:, fi, hosl],
                                    start=(fi == 0),
                                    stop=(fi == G - 1),
                                )
                        hsl = bass.ts(hf, half_w)
                        if g == 0:
                            nc.vector.tensor_scalar_mul(ot[:, hsl], po[:], gsl)
                        else:
                            nc.vector.scalar_tensor_tensor(
                                ot[:, hsl],
                                po[:],
                                gsl,
                                prev[:, hsl],
                                mybir.AluOpType.mult,
                                mybir.AluOpType.add,
                            )
                    dst = out_d if g == n_groups - 1 else part_d
                    nc.sync.dma_start(dst[ctsl, :], ot[:])
    nc.finalize()
    return nc


def _route(x, gate_w):
    """Host gate: top-2 + softmax.  Returns (idx per expert, weight per expert)."""
    xs = x.reshape(-1, x.shape[-1])
    logits = xs.astype(np.float32) @ gate_w.astype(np.float32)  # [T, E]
    # top-2 (ties broken by lower index, matching jax.lax.top_k)
    e1 = np.argmax(logits, axis=1)
    l1 = logits[np.arange(len(logits)), e1]
    masked = logits.copy()
    masked[np.arange(len(logits)), e1] = -np.inf
    e2 = np.argmax(masked, axis=1)
    l2 = masked[np.arange(len(logits)), e2]
    # softmax over the two logits
    w_hi = 1.0 / (1.0 + np.exp(l2 - l1))
    w_lo = 1.0 - w_hi
    idxs, gws = [], []
    for e in range(E):
        sel1 = e1 == e
        sel2 = e2 == e
        idx = np.nonzero(sel1 | sel2)[0]
        w = np.where(sel1[idx], w_hi[idx], w_lo[idx]).astype(np.float32)
        idxs.append(idx)
        gws.append(w)
    return xs, idxs, gws


def kernel(x, gate_w, w1, w3, w2):
    x = np.asarray(x)
    gate_w = np.asarray(gate_w)
    w1 = np.asarray(w1)
    w3 = np.asarray(w3)
    w2 = np.asarray(w2)

    xs, idxs, gws = _route(x, gate_w)
    T = xs.shape[0]
    max_load = max(len(i) for i in idxs)
    C = max(384, int(math.ceil(max_load / 384.0)) * 384)

    key = ("k", C)
    if key not in _kernel_cache:
        _kernel_cache[key] = build_expert_kernel(C)
    nc = _kernel_cache[key]

    in_maps = []
    for e in range(E):
        n_e = len(idxs[e])
        xt = np.zeros((H, C), np.float32)
        xt[:, :n_e] = xs[idxs[e]].T
        gwpad = np.zeros(C, np.float32)
        gwpad[:n_e] = gws[e]
        gws_arr = np.ascontiguousarray(gwpad.reshape(C // P, P).T)
        in_maps.append(
            {
                "xt": xt,
                "w1": np.ascontiguousarray(w1[e], dtype=np.float32),
                "w3": np.ascontiguousarray(w3[e], dtype=np.float32),
                "w2b": np.ascontiguousarray(w2[e]).astype(ml_dtypes.bfloat16),
                "gws": gws_arr,
            }
        )

    global LAST_RESULT
    if TRACE:
        try:
            res = run_bass_kernel_spmd(
                nc,
                in_maps,
                core_ids=list(range(E)),
                trace=True,
                trace_cores=list(range(E)),
            )
        except Exception as exc:
            import traceback

            print("TRACE FAILED:", exc)
            traceback.print_exc()
            res = run_bass_kernel_spmd(nc, in_maps, core_ids=list(range(E)))
    else:
        res = run_bass_kernel_spmd(nc, in_maps, core_ids=list(range(E)))
    LAST_RESULT = res
    results = res.results

    out_flat = np.zeros((T, H), np.float32)
    for e in range(E):
        n_e = len(idxs[e])
        out_flat[idxs[e]] += results[e]["out"][:n_e]
    return out_flat.reshape(x.shape)


# revision 7
# speedup vs baseline: 1.2592x; 1.0106x over previous
"""Mistral MoE layer (H=2048, F=8192, E=8, top-2) on 8 Trainium2 NeuronCores.

Strategy (expert parallelism, per the sharding hint):
  - Host computes the (tiny) gate: logits = x @ gate_w, top-2, softmax.
    This is 0.004% of the FLOPs; the expert FFNs dominate.
  - Host "all-to-all dispatch": tokens are gathered per expert into a
    padded capacity buffer of C tokens (C = max expert load rounded up).
  - Core e runs expert e's SwiGLU FFN over its C tokens:
        y = (silu(x @ w1) * (x @ w3)) @ w2, scaled per-token by the
    combined gate weight.
  - Host "combine": scatter-add each expert's rows back into the output.

Device kernel (per core):
  Stage 1 computes hT/uT = w1/w3-projections in *transposed* form
  [F, C] so stage 2 can use them directly as the stationary matmul
  operand without any on-device transpose.  Matmuls run as float32r
  (FP22 truncated fp32 — full 78.6 TF/s PE rate at moving dim >= 256).
  yT = silu(hT) * uT is stored bf16; stage 2 (down-projection) runs
  bf16 x bf16.  F is processed in groups of G=8 f-tiles; each group's
  partial down-projection output is accumulated via a DRAM bounce
  buffer: out_g = psum * gate_w + out_{g-1} (fused DVE op).
"""

import math

import numpy as np
import ml_dtypes

import concourse.bass as bass
import concourse.mybir as mybir
import concourse.tile as tile
from concourse import bacc
from concourse.bass_utils import run_bass_kernel_spmd

P = 128
H = 2048
F = 8192
E = 8
TOP_K = 2

_kernel_cache: dict = {}

# Test-harness knobs (ignored in normal use): when TRACE is true, the SPMD
# run captures an NTFF profile and the BassKernelResults lands in LAST_RESULT.
TRACE = False
LAST_RESULT = None


def build_expert_kernel(C, H_=H, F_=F, c_chunk=384, ho_chunk=512, G=8):
    """One expert's SwiGLU FFN over C tokens; returns finalized Bacc."""
    f32 = mybir.dt.float32
    f32r = mybir.dt.float32r
    bf16 = mybir.dt.bfloat16

    n_hh = H_ // P          # contraction tiles over hidden dim (stage 1)
    n_f = F_ // P           # f tiles
    n_ct = C // P           # token tiles (stage 2 output partitions)
    n_cch = C // c_chunk    # moving-dim chunks over tokens (stage 1)
    n_ho = H_ // ho_chunk   # moving-dim chunks over hidden dim (stage 2)
    n_groups = n_f // G
    assert C % c_chunk == 0 and C % P == 0 and F_ % (G * P) == 0

    nc = bacc.Bacc("TRN2", target_bir_lowering=False, debug=False)
    xt_d = nc.dram_tensor("xt", [H_, C], f32r, kind="ExternalInput")
    w1_d = nc.dram_tensor("w1", [H_, F_], f32r, kind="ExternalInput")
    w3_d = nc.dram_tensor("w3", [H_, F_], f32r, kind="ExternalInput")
    w2_d = nc.dram_tensor("w2b", [F_, H_], bf16, kind="ExternalInput")
    gw_d = nc.dram_tensor("gws", [P, n_ct], f32, kind="ExternalInput")
    out_d = nc.dram_tensor("out", [C, H_], f32, kind="ExternalOutput")

    xt_r = xt_d[:, :].rearrange("(ho hi) c -> hi ho c", hi=P)
    w1_r = w1_d[:, :].rearrange("(ho hi) f -> hi ho f", hi=P)
    w3_r = w3_d[:, :].rearrange("(ho hi) f -> hi ho f", hi=P)

    with tile.TileContext(nc) as tc:
        with (
            tc.tile_pool(name="persist", bufs=1) as persist,
            tc.tile_pool(name="wpool", bufs=2) as wpool,
            tc.tile_pool(name="ypool", bufs=1) as ypool,
            tc.tile_pool(name="spool", bufs=2) as spool,
            tc.tile_pool(name="opool", bufs=2) as opool,
            tc.tile_pool(name="psum", bufs=1, space="PSUM") as psum,
            tc.tile_pool(name="dram", bufs=1, space="DRAM") as dram,
        ):
            xt_s = persist.tile([P, n_hh, C], f32r, name="xt_s")
            for hh in range(n_hh):
                nc.sync.dma_start(xt_s[:, hh, :], xt_r[:, hh, :])
            gw_s = persist.tile([P, n_ct], f32, name="gw_s")
            nc.sync.dma_start(gw_s[:], gw_d[:, :])
            part_d = dram.tile([C, H_], f32, name="part")

            for g in range(n_groups):
                # ---- stage 1: yT[f_tile, :] for the G f-tiles of this group
                yt = ypool.tile([P, G, C], bf16, tag="yt", name="yt")
                for fi in range(G):
                    f = g * G + fi
                    fsl = bass.ts(f, P)
                    w1_t = wpool.tile([P, n_hh, P], f32r, tag="w1t", name="w1_t")
                    nc.sync.dma_start(w1_t[:], w1_r[:, :, fsl])
                    w3_t = wpool.tile([P, n_hh, P], f32r, tag="w3t", name="w3_t")
                    nc.sync.dma_start(w3_t[:], w3_r[:, :, fsl])
                    for ci in range(n_cch):
                        csl = bass.ts(ci, c_chunk)
                        ph = psum.tile([P, c_chunk], f32, tag="ph", bufs=2, name="ph")
                        pu = psum.tile([P, c_chunk], f32, tag="pu", bufs=2, name="pu")
                        for hh in range(n_hh):
                            nc.tensor.matmul(
                                ph[:],
                                w1_t[:, hh, :],
                                xt_s[:, hh, csl],
                                start=(hh == 0),
                                stop=(hh == n_hh - 1),
                            )
                        for hh in range(n_hh):
                            nc.tensor.matmul(
                                pu[:],
                                w3_t[:, hh, :],
                                xt_s[:, hh, csl],
                                start=(hh == 0),
                                stop=(hh == n_hh - 1),
                            )
                        sl = spool.tile([P, c_chunk], f32, tag="sl", name="sl")
                        nc.scalar.activation(
                            sl[:], ph[:], mybir.ActivationFunctionType.Silu
                        )
                        nc.vector.tensor_tensor(
                            yt[:, fi, csl], sl[:], pu[:], mybir.AluOpType.mult
                        )

                # ---- stage 2: partial down-projection for this group
                w2_t = wpool.tile([P, G, H_], bf16, tag="w2t", bufs=1, name="w2_t")
                for fi in range(G):
                    f = g * G + fi
                    nc.sync.dma_start(w2_t[:, fi, :], w2_d[bass.ts(f, P), :])
                # two ho-half passes so the PSUM tile is 2 banks and can be
                # double-buffered (next ct's matmuls overlap this ct's DVE read)
                n_half = n_ho // 2 if n_ho >= 2 else 1
                half_w = n_half * ho_chunk
                for ct in range(n_ct):
                    ctsl = bass.ts(ct, P)
                    ot = opool.tile([P, H_], f32, tag="ot", name="ot")
                    gsl = gw_s[:, ct : ct + 1]
                    prev = None
                    if g > 0:
                        prev = opool.tile([P, H_], f32, tag="prev", name="prev")
                        nc.sync.dma_start(prev[:], part_d[ctsl, :])
                    for hf in range(n_ho // n_half):
                        po = psum.tile([P, half_w], f32, tag="po", bufs=2, name="po")
                        for fi in range(G):
                            for ho in range(n_half):
                                hosl = bass.ts(hf * n_half + ho, ho_chunk)
                                nc.tensor.matmul(
                                    po[:, bass.ts(ho, ho_chunk)],
                                    yt[:, fi, ctsl],
                                    w2_t[:, fi, hosl],
                                    start=(fi == 0),
                                    stop=(fi == G - 1),
                                )
                        hsl = bass.ts(hf, half_w)
                        if g == 0:
                            nc.vector.tensor_scalar_mul(ot[:, hsl], po[:], gsl)
                        else:
                            nc.vector.scalar_tensor_tensor(
                                ot[:, hsl],
                                po[:],
                                gsl,
                                prev[:, hsl],
                                mybir.AluOpType.mult,
                                mybir.AluOpType.add,
                            )
                    dst = out_d if g == n_groups - 1 else part_d
                    nc.sync.dma_start(dst[ctsl, :], ot[:])
    nc.finalize()
    return nc


def _route(x, gate_w):
    """Host gate: top-2 + softmax.  Returns (idx per expert, weight per expert)."""
    xs = x.reshape(-1, x.shape[-1])
    logits = xs.astype(np.float32) @ gate_w.astype(np.float32)  # [T, E]
    # top-2 (ties broken by lower index, matching jax.lax.top_k)
    e1 = np.argmax(logits, axis=1)
    l1 = logits[np.arange(len(logits)), e1]
    masked = logits.copy()
    masked[np.arange(len(logits)), e1] = -np.inf
    e2 = np.argmax(masked, axis=1)
    l2 = masked[np.arange(len(logits)), e2]
    # softmax over the two logits
    w_hi = 1.0 / (1.0 + np.exp(l2 - l1))
    w_lo = 1.0 - w_hi
    idxs, gws = [], []
    for e in range(E):
        sel1 = e1 == e
        sel2 = e2 == e
        idx = np.nonzero(sel1 | sel2)[0]
        w = np.where(sel1[idx], w_hi[idx], w_lo[idx]).astype(np.float32)
        idxs.append(idx)
        gws.append(w)
    return xs, idxs, gws


def kernel(x, gate_w, w1, w3, w2):
    x = np.asarray(x)
    gate_w = np.asarray(gate_w)
    w1 = np.asarray(w1)
    w3 = np.asarray(w3)
    w2 = np.asarray(w2)

    xs, idxs, gws = _route(x, gate_w)
    T = xs.shape[0]
    max_load = max(len(i) for i in idxs)
    C = max(384, int(math.ceil(max_load / 384.0)) * 384)

    key = ("k", C)
    if key not in _kernel_cache:
        _kernel_cache[key] = build_expert_kernel(C)
    nc = _kernel_cache[key]

    in_maps = []
    for e in range(E):
        n_e = len(idxs[e])
        xt = np.zeros((H, C), np.float32)
        xt[:, :n_e] = xs[idxs[e]].T
        gwpad = np.zeros(C, np.float32)
        gwpad[:n_e] = gws[e]
        gws_arr = np.ascontiguousarray(gwpad.reshape(C // P, P).T)
        in_maps.append(
            {
                "xt": xt,
                "w1": np.ascontiguousarray(w1[e], dtype=np.float32),
                "w3": np.ascontiguousarray(w3[e], dtype=np.float32),
                "w2b": np.ascontiguousarray(w2[e]).astype(ml_dtypes.bfloat16),
                "gws": gws_arr,
            }
        )

    global LAST_RESULT
    if TRACE:
        try:
            res = run_bass_kernel_spmd(
                nc,
                in_maps,
                core_ids=list(range(E)),
                trace=True,
                trace_cores=list(range(E)),
            )
        except Exception as exc:
            import traceback

            print("TRACE FAILED:", exc)
            traceback.print_exc()
            res = run_bass_kernel_spmd(nc, in_maps, core_ids=list(range(E)))
    else:
        res = run_bass_kernel_spmd(nc, in_maps, core_ids=list(range(E)))
    LAST_RESULT = res
    results = res.results

    out_flat = np.zeros((T, H), np.float32)
    for e in range(E):
        n_e = len(idxs[e])
        out_flat[idxs[e]] += results[e]["out"][:n_e]
    return out_flat.reshape(x.shape)
